# revision 1
# baseline (speedup 1.0000x reference)
"""Trainium2 Bass kernel for CharacterNet segment-mean + FC (segment_reduce).

Reference computation (per batch row b of 32):
  x = all_encoder_layers[layer_index][b]          # (512, 768)
  for t in 0..255: mean_t = mean(x[token_map[b,t]:token_map[b,t+1]])
  ote[b*256+t] = mean_t                           # (8192, 768) output 2
  rep = ote @ fc_w.T + fc_b                       # (8192, 768) output 1

Strategy: data-parallel over batch across 8 NeuronCores (4 rows/core).
The segment mean is a matmul with a one-hot-per-row selection matrix
SelT (512, 256), SelT[s, t] = (seg(s)==t) / count(seg(s)), built on
device from two tiny per-position index vectors with a single
tensor_scalar op per (128,256) chunk.  Stage 1 computes meanT = x.T @
SelT (H on partitions) so stage 2 (the FC) can consume it directly as
the stationary operand; the natural-orientation ote output is produced
with PE transposes.  Matmuls run as float32r (TF32-precision, full PE
rate); everything else stays fp32.
"""

import os
import numpy as np

import concourse.bass as bass
import concourse.bacc as bacc
import concourse.mybir as mybir
import concourse.tile as tile
from concourse import masks
from concourse.bass_utils import run_bass_kernel_spmd

N_CORES = 8
B, S, H, T = 32, 512, 768, 256
B_LOC = B // N_CORES          # 4 batch rows per core
NS = S // 128                 # 4 s-chunks per row
NJ = B_LOC * NS               # 16 (128,768) x chunks per core
NH = H // 128                 # 6 h-chunks
NB2 = 384                     # stage-2 N tile (two per 768)

F32 = mybir.dt.float32
# float32r = TF32-precision matmul at full PE rate (1 cyc/row for N>=256)
# vs plain fp32 at 4 cyc/row.  Switchable for accuracy fallback.
MM_DT = mybir.dt.float32r if os.environ.get("KERNEL_MM_DT", "f32r") == "f32r" else F32

# tunables (model-searched): engine for each PSUM-evict copy class,
# direct PSUM->DRAM DMA for outputs, psum pool sizes
OPT = {
    "m_copy": "vector",      # meanT psum->sbuf: vector | scalar
    "ote_copy": "vector",    # transpose psum->sbuf: vector | scalar
    "rep_copy": "scalar",    # stage2 psum->sbuf: vector | scalar | dma
    "ote_dma_direct": False, # DMA each transpose psum straight to DRAM
    "p1": 3, "pt": 2, "p2": 3,
    "in_dma": "sync", "out_dma": "scalar",
    "w_after": 3,            # emit fc_w DMAs after this many x2 DMAs
    "bias_mm": True,         # emit the K=1 bias matmuls (False when fc_b==0)
    "x_split_first": True,   # first x2 pair as two 384KB DMAs (earlier PE start)
    "tr_f32r": False,        # PE transposes in f32r (1.5 vs 2 cyc/row)
    "out_split": True,       # output DMAs per row-chunk (1.1us) vs per-b pair
}


def _copy(nc, engine, dst, src_):
    if engine == "scalar":
        nc.scalar.copy(dst, src_)
    else:
        nc.vector.tensor_copy(dst, src_)


def _f32(ap):
    # view an MM_DT tile as plain fp32 (exact datapath, e.g. PE transpose)
    return ap.bitcast(F32) if MM_DT != F32 else ap


def _r(ap):
    # view a fp32 DRAM region as MM_DT for a byte-copy DMA into an MM_DT tile
    return ap.bitcast(MM_DT) if MM_DT != F32 else ap


def build_kernel(reps: int = 1, loop: bool = False,
                 bias_mm: bool | None = None) -> bass.Bass:
    if bias_mm is not None:
        OPT["bias_mm"] = bias_mm
    nc = bacc.Bacc("TRN2", target_bir_lowering=False, debug=False,
                   num_devices=N_CORES)

    x_d = nc.dram_tensor("x", (NJ * 128, H), F32, kind="ExternalInput")
    # packed aux: cols 0..15 = seg, 16..31 = inv  (128, 32)
    aux_d = nc.dram_tensor("selaux", (128, 2 * NJ), F32, kind="ExternalInput")
    fcw_d = nc.dram_tensor("fcwT", (H, H), F32, kind="ExternalInput")
    # packed bias row: [0:H]=fc_b, [H:H+128]=ones
    bias_d = nc.dram_tensor("biasaux", (1, H + 128), F32, kind="ExternalInput")
    identr_d = nc.dram_tensor("identr", (128, 128), F32, kind="ExternalInput")
    rep_d = nc.dram_tensor("rep", (B_LOC * T, H), F32, kind="ExternalOutput")
    ote_d = nc.dram_tensor("ote", (B_LOC * T, H), F32, kind="ExternalOutput")

    # paired-row-chunk views for 768 KB DMAs: [j0][p, q, h] = t[(2*j0+q)*128+p, h]
    x_v = x_d.rearrange("(a q p) h -> a p q h", q=2, p=128)
    rep_v = rep_d.rearrange("(a q p) h -> a p q h", q=2, p=128)
    ote_v = ote_d.rearrange("(a q p) h -> a p q h", q=2, p=128)

    with tile.TileContext(nc) as tc:
        with (
            tc.tile_pool(name="const", bufs=1) as cpool,
            tc.tile_pool(name="xp", bufs=1) as xpool,
            tc.tile_pool(name="selp", bufs=1) as selpool,
            tc.tile_pool(name="mp", bufs=1) as mpool,
            tc.tile_pool(name="wp", bufs=1) as wpool,
            tc.tile_pool(name="ob", bufs=2) as opool,
            tc.tile_pool(name="p1", bufs=OPT["p1"], space="PSUM") as p1pool,
            tc.tile_pool(name="pt", bufs=OPT["pt"], space="PSUM") as ptpool,
            tc.tile_pool(name="p2", bufs=OPT["p2"], space="PSUM") as p2pool,
        ):
            # one-time constants
            iota_t = cpool.tile([128, T], F32, tag="iota")
            nc.gpsimd.iota(iota_t[:], pattern=[[1, T]], base=0,
                           channel_multiplier=0,
                           allow_small_or_imprecise_dtypes=True)
            if OPT["tr_f32r"] and MM_DT != F32:
                ident = cpool.tile([128, 128], MM_DT, tag="ident")
                nc.sync.dma_start(ident[:], _r(identr_d[:]))
                _tr = lambda ap: ap
                TR_DT = MM_DT
            else:
                ident = cpool.tile([128, 128], F32, tag="ident")
                masks.make_identity(nc, ident[:])
                _tr = _f32
                TR_DT = F32

            def emit_rep():
                aux_sb = cpool.tile([128, 2 * NJ], F32, tag="aux")
                bias_sb = cpool.tile([1, H + 128], MM_DT, tag="bias")
                idma = getattr(nc, OPT["in_dma"])
                idma.dma_start(aux_sb[:], aux_d[:])
                idma.dma_start(bias_sb[:], _r(bias_d[:]))
                fcb_sb = bias_sb[:1, 0:H]
                ones = bias_sb[:1, H:H + 128]

                w_sb, x2_sb = [], []

                def emit_w():
                    for k in range(NH):
                        w = wpool.tile([128, H], MM_DT, tag=f"w{k}")
                        idma.dma_start(w[:],
                                       _r(fcw_d[k * 128:(k + 1) * 128, :]))
                        w_sb.append(w)

                for j0 in range(NJ // 2):
                    if j0 == OPT["w_after"]:
                        emit_w()
                    x2 = xpool.tile([128, 2 * H], MM_DT, tag=f"x{j0}")
                    if j0 == 0 and OPT["x_split_first"]:
                        for q in range(2):
                            idma.dma_start(
                                x2[:, q * H:(q + 1) * H],
                                _r(x_d[q * 128:(q + 1) * 128, :]))
                    else:
                        idma.dma_start(
                            x2[:].rearrange("p (q h) -> p q h", q=2),
                            _r(x_v[j0]))
                    x2_sb.append(x2)
                if OPT["w_after"] >= NJ // 2:
                    emit_w()

                def x_chunk(j, mh):
                    # (128,128) stationary slice of wp-token chunk j, h-chunk mh
                    q, j0 = j % 2, j // 2
                    o = q * H + mh * 128
                    return x2_sb[j0][:, o:o + 128]

                sel_sb = []
                for j in range(NJ):
                    sel = selpool.tile([128, T], MM_DT, tag=f"s{j}")
                    # Sel^T chunk: (s==seg member of segment t) * 1/count
                    nc.vector.tensor_scalar(
                        sel[:], iota_t[:],
                        aux_sb[:, j:j + 1], aux_sb[:, NJ + j:NJ + j + 1],
                        op0=mybir.AluOpType.is_equal,
                        op1=mybir.AluOpType.mult)
                    sel_sb.append(sel)

                for b in range(B_LOC):
                    # stage 1: meanT[b] (768, 256) = x[b].T @ SelT[b]
                    mb = []
                    for mh in range(NH):
                        m = mpool.tile([128, T], MM_DT, tag=f"m{b}_{mh}")
                        ps = p1pool.tile([128, T], F32, tag="ps1")
                        for ks in range(NS):
                            j = b * NS + ks
                            nc.tensor.matmul(
                                ps[:],
                                x_chunk(j, mh),
                                sel_sb[j][:],
                                start=(ks == 0), stop=(ks == NS - 1))
                        _copy(nc, OPT["m_copy"], m[:], ps[:])
                        mb.append(m)

                    # ote rows of b: transpose meanT chunks to natural layout
                    odma = getattr(nc, OPT["out_dma"])
                    if OPT["ote_dma_direct"]:
                        for tq in range(2):
                            for mh in range(NH):
                                pt = ptpool.tile([128, 128], TR_DT, tag="pst")
                                nc.tensor.transpose(
                                    pt[:],
                                    _tr(mb[mh][:, tq * 128:(tq + 1) * 128]),
                                    ident[:])
                                r0 = (b * 2 + tq) * 128
                                odma.dma_start(
                                    ote_d[r0:r0 + 128,
                                          mh * 128:(mh + 1) * 128],
                                    _r(pt[:]) if TR_DT != F32 else pt[:])
                    else:
                        osb = opool.tile([128, 2 * H], F32, tag="osb")
                        for tq in range(2):
                            for mh in range(NH):
                                pt = ptpool.tile([128, 128], TR_DT, tag="pst")
                                nc.tensor.transpose(
                                    pt[:],
                                    _tr(mb[mh][:, tq * 128:(tq + 1) * 128]),
                                    ident[:])
                                _copy(nc, OPT["ote_copy"],
                                      osb[:, tq * H + mh * 128:
                                          tq * H + (mh + 1) * 128], pt[:])
                        if OPT["out_split"]:
                            for tq in range(2):
                                r0 = (b * 2 + tq) * 128
                                odma.dma_start(ote_d[r0:r0 + 128, :],
                                               osb[:, tq * H:(tq + 1) * H])
                        else:
                            odma.dma_start(
                                ote_v[b],
                                osb[:].rearrange("p (q h) -> p q h", q=2))

                    # stage 2: rep rows of b = meanT.T @ fc_w.T + fc_b
                    rsb = (None if OPT["rep_copy"] == "dma"
                           else opool.tile([128, 2 * H], F32, tag="rsb"))
                    for tq in range(2):
                        for nh in range(2):
                            ps2 = p2pool.tile([128, NB2], F32, tag="ps2")
                            nsl = slice(nh * NB2, (nh + 1) * NB2)
                            for kh in range(NH):
                                nc.tensor.matmul(
                                    ps2[:],
                                    mb[kh][:, tq * 128:(tq + 1) * 128],
                                    w_sb[kh][:, nsl],
                                    start=(kh == 0),
                                    stop=(not OPT["bias_mm"]
                                          and kh == NH - 1))
                            if OPT["bias_mm"]:
                                nc.tensor.matmul(
                                    ps2[:], ones[:1, :], fcb_sb[:1, nsl],
                                    start=False, stop=True)
                            if OPT["rep_copy"] == "dma":
                                r0 = (b * 2 + tq) * 128
                                odma.dma_start(
                                    rep_d[r0:r0 + 128, nsl], ps2[:])
                            else:
                                _copy(nc, OPT["rep_copy"],
                                      rsb[:, tq * H + nh * NB2:
                                          tq * H + (nh + 1) * NB2], ps2[:])
                    if OPT["rep_copy"] != "dma":
                        if OPT["out_split"]:
                            for tq in range(2):
                                r0 = (b * 2 + tq) * 128
                                odma.dma_start(rep_d[r0:r0 + 128, :],
                                               rsb[:, tq * H:(tq + 1) * H])
                        else:
                            odma.dma_start(
                                rep_v[b],
                                rsb[:].rearrange("p (q h) -> p q h", q=2))

            if loop and reps > 1:
                with tc.For_i(0, reps, 1,
                              hint_engines=(mybir.EngineType.PE,)):
                    emit_rep()
            else:
                for _ in range(reps):
                    emit_rep()

    nc.compile()
    return nc


def _host_prep(all_encoder_layers, token_map, fc_w, fc_b, layer_index):
    """Slice the chosen layer and build per-core input maps."""
    layer = int(np.asarray(layer_index))
    x_full = np.ascontiguousarray(
        np.asarray(all_encoder_layers)[layer], dtype=np.float32)  # (B, S, H)
    tm = np.asarray(token_map).astype(np.int64)                   # (B, T+1)

    pos = np.arange(S)
    seg = np.empty((B, S), dtype=np.int64)
    for b in range(B):
        seg[b] = np.searchsorted(tm[b], pos, side="right") - 1
    valid = pos[None, :] < tm[:, -1:]
    seg = np.where(valid, np.clip(seg, 0, T - 1), T)              # (B, S)
    counts = (tm[:, 1:] - tm[:, :-1]).astype(np.float32)          # (B, T)
    inv = np.zeros((B, S), dtype=np.float32)
    bb = np.arange(B)[:, None]
    iv = seg < T
    inv[iv] = (np.float32(1.0) /
               counts[np.broadcast_to(bb, seg.shape)[iv], seg[iv]])

    fcwT = np.ascontiguousarray(np.asarray(fc_w, dtype=np.float32).T)
    fcb = np.asarray(fc_b, dtype=np.float32).reshape(1, H)

    in_maps = []
    for c in range(N_CORES):
        bs = slice(c * B_LOC, (c + 1) * B_LOC)
        # (B_LOC, S) -> (128, NJ) with column j = b*NS + chunk
        seg_t = seg[bs].reshape(NJ, 128).T.astype(np.float32)
        inv_t = inv[bs].reshape(NJ, 128).T
        aux = np.ascontiguousarray(
            np.concatenate([seg_t, inv_t], axis=1))          # (128, 2*NJ)
        bias_aux = np.ascontiguousarray(np.concatenate(
            [fcb, np.ones((1, 128), np.float32)], axis=1))   # (1, H+128)
        in_maps.append({
            "x": np.ascontiguousarray(x_full[bs].reshape(NJ * 128, H)),
            "selaux": aux,
            "biasaux": bias_aux,
            "fcwT": fcwT,
            "identr": np.eye(128, dtype=np.float32),
        })
    return in_maps


class CachedRunner:
    """Jit/compile/load the bass program once; later calls are pure executes."""

    def __init__(self, nc, donate: bool = True):
        import jax
        from jax.sharding import Mesh, PartitionSpec
        from jax.experimental.shard_map import shard_map
        from concourse import bass2jax

        bass2jax.install_neuronx_cc_hook()
        self.nc = nc
        in_names, out_names, out_avals = [], [], []
        pname = nc.partition_id_tensor.name if nc.partition_id_tensor else None
        for alloc in nc.m.functions[0].allocations:
            if not isinstance(alloc, mybir.MemoryLocationSet):
                continue
            name = alloc.memorylocations[0].name
            if alloc.kind == "ExternalInput":
                if name != pname:
                    in_names.append(name)
            elif alloc.kind == "ExternalOutput":
                shape = tuple(alloc.tensor_shape)
                dtype = mybir.dt.np(alloc.dtype)
                out_names.append(name)
                out_avals.append(jax.core.ShapedArray(shape, dtype))
        self.in_names = list(in_names)
        self.out_names = out_names
        self.out_avals = out_avals
        n_params = len(in_names)
        n_outs = len(out_names)
        all_in_names = list(in_names) + list(out_names)
        if pname is not None:
            all_in_names.append(pname)
        donate_idx = tuple(range(n_params, n_params + n_outs)) if donate else ()

        def _body(*args):
            operands = list(args)
            if pname is not None:
                operands.append(bass2jax.partition_id_tensor())
            outs = bass2jax._bass_exec_p.bind(
                *operands,
                out_avals=tuple(out_avals),
                in_names=tuple(all_in_names),
                out_names=tuple(out_names),
                lowering_input_output_aliases=(),
                sim_require_finite=True,
                sim_require_nnan=True,
                nc=nc,
            )
            return tuple(outs)

        devices = jax.devices()[:N_CORES]
        mesh = Mesh(np.asarray(devices), ("core",))
        in_specs = (PartitionSpec("core"),) * (n_params + n_outs)
        out_specs = (PartitionSpec("core"),) * n_outs
        self.mesh = mesh
        self.sharding = jax.sharding.NamedSharding(mesh, PartitionSpec("core"))
        self.sharded = jax.jit(
            shard_map(_body, mesh=mesh, in_specs=in_specs,
                      out_specs=out_specs, check_rep=False),
            donate_argnums=donate_idx, keep_unused=True)
        self._dev_args = None

    def __call__(self, in_maps):
        concat_in = [
            np.concatenate([np.asarray(in_maps[c][n]) for c in range(N_CORES)], 0)
            for n in self.in_names]
        concat_zeros = [
            np.zeros((N_CORES * a.shape[0], *a.shape[1:]), a.dtype)
            for a in self.out_avals]
        out = self.sharded(*concat_in, *concat_zeros)
        return out  # list of jax arrays, concatenated over cores on axis 0

    def prepare(self, in_maps):
        """device_put all arguments once (requires donate=False runner)."""
        import jax
        concat_in = [
            np.concatenate([np.asarray(in_maps[c][n]) for c in range(N_CORES)], 0)
            for n in self.in_names]
        concat_zeros = [
            np.zeros((N_CORES * a.shape[0], *a.shape[1:]), a.dtype)
            for a in self.out_avals]
        self._dev_args = [jax.device_put(a, self.sharding)
                          for a in concat_in + concat_zeros]
        jax.block_until_ready(self._dev_args)

    def run_prepared(self):
        return self.sharded(*self._dev_args)

    def to_maps(self, out):
        return [
            {n: np.asarray(out[i]).reshape(N_CORES, *self.out_avals[i].shape)[c]
             for i, n in enumerate(self.out_names)}
            for c in range(N_CORES)]


_RUNNER_CACHE: dict = {}


def get_runner(reps: int = 1, loop: bool = False, donate: bool = True,
               bias_mm: bool = True) -> CachedRunner:
    key = (reps, loop, donate, bias_mm)
    if key not in _RUNNER_CACHE:
        _RUNNER_CACHE[key] = CachedRunner(
            build_kernel(reps, loop, bias_mm=bias_mm), donate)
    return _RUNNER_CACHE[key]


def kernel(all_encoder_layers, input_mask, token_map, fc_w, fc_b, layer_index):
    in_maps = _host_prep(all_encoder_layers, token_map, fc_w, fc_b, layer_index)
    bias_mm = bool(np.any(np.asarray(fc_b)))
    runner = get_runner(1, bias_mm=bias_mm)
    out = runner(in_maps)
    idx = {n: i for i, n in enumerate(runner.out_names)}
    rep = np.asarray(out[idx["rep"]])
    ote = np.asarray(out[idx["ote"]])
    return rep.astype(np.float32), ote.astype(np.float32)



# revision 3
# speedup vs baseline: 4.8487x; 4.8487x over previous
"""Trainium2 Bass kernel for CharacterNet segment-mean + FC (segment_reduce).

Reference computation (per batch row b of 32):
  x = all_encoder_layers[layer_index][b]          # (512, 768)
  for t in 0..255: mean_t = mean(x[token_map[b,t]:token_map[b,t+1]])
  ote[b*256+t] = mean_t                           # (8192, 768) output 2
  rep = ote @ fc_w.T + fc_b                       # (8192, 768) output 1

Strategy: data-parallel over batch across 8 NeuronCores (4 rows/core).
The segment mean is a matmul with a one-hot-per-row selection matrix
SelT (512, 256), SelT[s, t] = (seg(s)==t) / count(seg(s)), built on
device from two tiny per-position index vectors with one tensor_scalar
op per (128,256) chunk.  All heavy tensors move over HBM as bf16
(host casts inputs / upcasts outputs), halving DMA traffic vs fp32;
matmuls are bf16 at full PE rate with fp32 PSUM accumulation.

Both outputs are produced in "transposed" partition-major DRAM layouts
chosen so no on-device transposes are needed (the host does the final
cheap re-layout):
  stage 1: meanT[b] (768h, 256t) = x[b].T @ SelT[b]   -> ote as-is
  stage 2: repT[b] (768n, 256t)  = fcwT.T @ meanT[b]  -> fc_b folded
           into the PSUM->SBUF eviction as a per-partition bias add.
"""

import numpy as np

import concourse.bass as bass
import concourse.bacc as bacc
import concourse.mybir as mybir
import concourse.tile as tile

N_CORES = 8
B, S, H, T = 32, 512, 768, 256
B_LOC = B // N_CORES          # 4 batch rows per core
NS = S // 128                 # 4 s-chunks per row
NJ = B_LOC * NS               # 16 (128-row) s-chunks per core
NH = H // 128                 # 6 h-chunks
WB = NH * T                   # 1536 output cols per batch row (both outs)

F32 = mybir.dt.float32
BF16 = mybir.dt.bfloat16
NP_BF16 = mybir.dt.np(BF16)

# tunables
OPT = {
    "x_dmas": 4,          # input x split into this many DMAs (1/2/4)
    "w_after": 2,         # emit fc_w DMA after this many x DMAs
    "m_copy": "vector",   # stage-1 psum evict engine: vector | scalar
    "rep_copy": "scalar", # stage-2 psum evict (+bias) engine
    "ote_group": 2,       # batch rows per ote output DMA (1/2/4)
    "rep_group": 2,       # batch rows per rep output DMA (1/2/4)
    "stagger": 1,         # sw-pipeline: s2(b) issued after s1(b+stagger)
    "p1": 3, "p2": 3,     # psum pool bufs
    "xbufs": 2, "wbufs": 2, "selbufs": 2, "mbufs": 2, "rbufs": 2,
    "in_dma": "sync", "out_dma": "scalar",
}


def build_kernel(reps: int = 1, loop: bool = False) -> bass.Bass:
    nc = bacc.Bacc("TRN2", target_bir_lowering=False, debug=False,
                   num_devices=N_CORES)

    # x: [p, j*H + h] = x[j*128+p, h], j = b*NS + ks  (24KB/partition)
    x_d = nc.dram_tensor("x", (128, NJ * H), BF16, kind="ExternalInput")
    # fcwT: [p, k*H + n] = fc_w[n, k*128+p]
    fcw_d = nc.dram_tensor("fcwT", (128, NH * H), BF16, kind="ExternalInput")
    # packed aux: cols 0..15 = seg id, 16..31 = 1/count  (128, 32)
    aux_d = nc.dram_tensor("selaux", (128, 2 * NJ), F32, kind="ExternalInput")
    # fc_b as per-partition columns: [p, nh] = fc_b[nh*128+p]
    fcb_d = nc.dram_tensor("fcbcol", (128, NH), F32, kind="ExternalInput")
    # outputs, transposed layouts: [p, (b*NH+c)*T + t]
    rep_d = nc.dram_tensor("rep", (128, B_LOC * WB), BF16,
                           kind="ExternalOutput")
    ote_d = nc.dram_tensor("ote", (128, B_LOC * WB), BF16,
                           kind="ExternalOutput")

    with tile.TileContext(nc) as tc:
        with (
            tc.tile_pool(name="const", bufs=1) as cpool,
            tc.tile_pool(name="aux", bufs=2) as apool,
            tc.tile_pool(name="xp", bufs=OPT["xbufs"]) as xpool,
            tc.tile_pool(name="wp", bufs=OPT["wbufs"]) as wpool,
            tc.tile_pool(name="selp", bufs=OPT["selbufs"]) as selpool,
            tc.tile_pool(name="mp", bufs=OPT["mbufs"]) as mpool,
            tc.tile_pool(name="rp", bufs=OPT["rbufs"]) as rpool,
            tc.tile_pool(name="p1", bufs=OPT["p1"], space="PSUM") as p1pool,
            tc.tile_pool(name="p2", bufs=OPT["p2"], space="PSUM") as p2pool,
        ):
            # one-time constant: iota 0..T-1 along free dim, all partitions
            iota_t = cpool.tile([128, T], F32, tag="iota")
            nc.gpsimd.iota(iota_t[:], pattern=[[1, T]], base=0,
                           channel_multiplier=0,
                           allow_small_or_imprecise_dtypes=True)

            idma = getattr(nc, OPT["in_dma"])
            odma = getattr(nc, OPT["out_dma"])

            def emit_rep():
                aux_sb = apool.tile([128, 2 * NJ], F32, tag="aux")
                fcb_sb = apool.tile([128, NH], F32, tag="fcb")
                idma.dma_start(aux_sb[:], aux_d[:])
                idma.dma_start(fcb_sb[:], fcb_d[:])

                # input DMAs: x in x_dmas chunks, fc_w staggered between
                x_sb = xpool.tile([128, NJ * H], BF16, tag="x")
                w_sb = wpool.tile([128, NH * H], BF16, tag="w")
                nx = OPT["x_dmas"]
                xw = NJ * H // nx
                for i in range(nx):
                    if i == OPT["w_after"]:
                        idma.dma_start(w_sb[:], fcw_d[:])
                    idma.dma_start(x_sb[:, i * xw:(i + 1) * xw],
                                   x_d[:, i * xw:(i + 1) * xw])
                if OPT["w_after"] >= nx:
                    idma.dma_start(w_sb[:], fcw_d[:])

                # SelT chunks: sel[:, j*T+t] = (seg(s)==t)/count, s=j*128+p
                sel_sb = selpool.tile([128, NJ * T], BF16, tag="sel")
                for j in range(NJ):
                    nc.vector.tensor_scalar(
                        sel_sb[:, j * T:(j + 1) * T], iota_t[:],
                        aux_sb[:, j:j + 1], aux_sb[:, NJ + j:NJ + j + 1],
                        op0=mybir.AluOpType.is_equal,
                        op1=mybir.AluOpType.mult)

                m_sb = mpool.tile([128, B_LOC * WB], BF16, tag="m")
                r_sb = rpool.tile([128, B_LOC * WB], BF16, tag="r")

                def s1(b):
                    # meanT[b] (768, 256) = x[b].T @ SelT[b]
                    for mh in range(NH):
                        ps = p1pool.tile([128, T], F32, tag="ps1")
                        for ks in range(NS):
                            j = b * NS + ks
                            nc.tensor.matmul(
                                ps[:],
                                x_sb[:, j * H + mh * 128:
                                     j * H + (mh + 1) * 128],
                                sel_sb[:, j * T:(j + 1) * T],
                                start=(ks == 0), stop=(ks == NS - 1))
                        dst = m_sb[:, b * WB + mh * T:b * WB + (mh + 1) * T]
                        if OPT["m_copy"] == "scalar":
                            nc.scalar.copy(dst, ps[:])
                        else:
                            nc.vector.tensor_copy(dst, ps[:])
                    g = OPT["ote_group"]
                    if (b + 1) % g == 0:
                        c0 = (b + 1 - g) * WB
                        odma.dma_start(ote_d[:, c0:(b + 1) * WB],
                                       m_sb[:, c0:(b + 1) * WB])

                def s2(b):
                    # repT[b] (768, 256) = fcwT.T @ meanT[b] + fc_b
                    for nh in range(NH):
                        ps = p2pool.tile([128, T], F32, tag="ps2")
                        for kh in range(NH):
                            nc.tensor.matmul(
                                ps[:],
                                w_sb[:, kh * H + nh * 128:
                                     kh * H + (nh + 1) * 128],
                                m_sb[:, b * WB + kh * T:b * WB + (kh + 1) * T],
                                start=(kh == 0), stop=(kh == NH - 1))
                        dst = r_sb[:, b * WB + nh * T:b * WB + (nh + 1) * T]
                        if OPT["rep_copy"] == "scalar":
                            nc.scalar.activation(
                                dst, ps[:],
                                mybir.ActivationFunctionType.Identity,
                                bias=fcb_sb[:, nh:nh + 1], scale=1.0)
                        else:
                            nc.vector.tensor_scalar(
                                dst, ps[:], fcb_sb[:, nh:nh + 1], None,
                                op0=mybir.AluOpType.add)
                    g = OPT["rep_group"]
                    if (b + 1) % g == 0:
                        c0 = (b + 1 - g) * WB
                        odma.dma_start(rep_d[:, c0:(b + 1) * WB],
                                       r_sb[:, c0:(b + 1) * WB])

                # software-pipelined issue order on the in-order PE
                st = OPT["stagger"]
                for b in range(B_LOC + st):
                    if b < B_LOC:
                        s1(b)
                    if b >= st:
                        s2(b - st)

            if loop and reps > 1:
                with tc.For_i(0, reps, 1,
                              hint_engines=(mybir.EngineType.PE,)):
                    emit_rep()
            else:
                for _ in range(reps):
                    emit_rep()

    nc.compile()
    return nc


def _host_prep(all_encoder_layers, token_map, fc_w, fc_b, layer_index):
    """Slice the chosen layer and build per-core bf16 input maps."""
    layer = int(np.asarray(layer_index))
    x_full = np.asarray(all_encoder_layers)[layer]                # (B, S, H)
    tm = np.asarray(token_map).astype(np.int64)                   # (B, T+1)

    pos = np.arange(S)
    seg = np.empty((B, S), dtype=np.int64)
    for b in range(B):
        seg[b] = np.searchsorted(tm[b], pos, side="right") - 1
    valid = pos[None, :] < tm[:, -1:]
    seg = np.where(valid, np.clip(seg, 0, T - 1), T)              # (B, S)
    counts = (tm[:, 1:] - tm[:, :-1]).astype(np.float32)          # (B, T)
    inv = np.zeros((B, S), dtype=np.float32)
    bb = np.arange(B)[:, None]
    iv = seg < T
    inv[iv] = (np.float32(1.0) /
               counts[np.broadcast_to(bb, seg.shape)[iv], seg[iv]])

    # fcwT partition-major: [p, k*H+n] = fc_w[n, k*128+p]
    fcwT = np.asarray(fc_w, dtype=np.float32).T.astype(NP_BF16)
    wt = np.ascontiguousarray(
        fcwT.reshape(NH, 128, H).transpose(1, 0, 2).reshape(128, NH * H))
    fcb_col = np.ascontiguousarray(
        np.asarray(fc_b, dtype=np.float32).reshape(NH, 128).T)

    in_maps = []
    for c in range(N_CORES):
        bs = slice(c * B_LOC, (c + 1) * B_LOC)
        # x partition-major: [p, j*H+h] = x[bs][j*128+p, h]
        xt = np.ascontiguousarray(
            x_full[bs].astype(NP_BF16).reshape(NJ, 128, H)
            .transpose(1, 0, 2).reshape(128, NJ * H))
        # (B_LOC, S) -> (128, NJ) with column j = b*NS + chunk
        seg_t = seg[bs].reshape(NJ, 128).T.astype(np.float32)
        inv_t = inv[bs].reshape(NJ, 128).T
        aux = np.ascontiguousarray(
            np.concatenate([seg_t, inv_t], axis=1))          # (128, 2*NJ)
        in_maps.append({
            "x": xt,
            "selaux": aux,
            "fcwT": wt,
            "fcbcol": fcb_col,
        })
    return in_maps


def _unpack_out(arr):
    """(N_CORES*128, B_LOC*WB) bf16 transposed layout -> (B*T, H) fp32."""
    a = np.asarray(arr).astype(np.float32)
    a = a.reshape(N_CORES, 128, B_LOC, NH, T)
    return np.ascontiguousarray(
        a.transpose(0, 2, 4, 3, 1).reshape(B * T, H))


class CachedRunner:
    """Jit/compile/load the bass program once; later calls are pure executes."""

    def __init__(self, nc, donate: bool = True):
        import jax
        from jax.sharding import Mesh, PartitionSpec
        from jax.experimental.shard_map import shard_map
        from concourse import bass2jax

        bass2jax.install_neuronx_cc_hook()
        self.nc = nc
        in_names, out_names, out_avals = [], [], []
        pname = nc.partition_id_tensor.name if nc.partition_id_tensor else None
        for alloc in nc.m.functions[0].allocations:
            if not isinstance(alloc, mybir.MemoryLocationSet):
                continue
            name = alloc.memorylocations[0].name
            if alloc.kind == "ExternalInput":
                if name != pname:
                    in_names.append(name)
            elif alloc.kind == "ExternalOutput":
                shape = tuple(alloc.tensor_shape)
                dtype = mybir.dt.np(alloc.dtype)
                out_names.append(name)
                out_avals.append(jax.core.ShapedArray(shape, dtype))
        self.in_names = list(in_names)
        self.out_names = out_names
        self.out_avals = out_avals
        n_params = len(in_names)
        n_outs = len(out_names)
        all_in_names = list(in_names) + list(out_names)
        if pname is not None:
            all_in_names.append(pname)
        donate_idx = tuple(range(n_params, n_params + n_outs)) if donate else ()

        def _body(*args):
            operands = list(args)
            if pname is not None:
                operands.append(bass2jax.partition_id_tensor())
            outs = bass2jax._bass_exec_p.bind(
                *operands,
                out_avals=tuple(out_avals),
                in_names=tuple(all_in_names),
                out_names=tuple(out_names),
                lowering_input_output_aliases=(),
                sim_require_finite=True,
                sim_require_nnan=True,
                nc=nc,
            )
            return tuple(outs)

        devices = jax.devices()[:N_CORES]
        mesh = Mesh(np.asarray(devices), ("core",))
        in_specs = (PartitionSpec("core"),) * (n_params + n_outs)
        out_specs = (PartitionSpec("core"),) * n_outs
        self.mesh = mesh
        self.sharding = jax.sharding.NamedSharding(mesh, PartitionSpec("core"))
        self.sharded = jax.jit(
            shard_map(_body, mesh=mesh, in_specs=in_specs,
                      out_specs=out_specs, check_rep=False),
            donate_argnums=donate_idx, keep_unused=True)
        self._dev_args = None

    def __call__(self, in_maps):
        concat_in = [
            np.concatenate([np.asarray(in_maps[c][n]) for c in range(N_CORES)], 0)
            for n in self.in_names]
        concat_zeros = [
            np.zeros((N_CORES * a.shape[0], *a.shape[1:]), a.dtype)
            for a in self.out_avals]
        out = self.sharded(*concat_in, *concat_zeros)
        return out  # list of jax arrays, concatenated over cores on axis 0

    def prepare(self, in_maps):
        """device_put all arguments once (requires donate=False runner)."""
        import jax
        concat_in = [
            np.concatenate([np.asarray(in_maps[c][n]) for c in range(N_CORES)], 0)
            for n in self.in_names]
        concat_zeros = [
            np.zeros((N_CORES * a.shape[0], *a.shape[1:]), a.dtype)
            for a in self.out_avals]
        self._dev_args = [jax.device_put(a, self.sharding)
                          for a in concat_in + concat_zeros]
        jax.block_until_ready(self._dev_args)

    def run_prepared(self):
        return self.sharded(*self._dev_args)


_RUNNER_CACHE: dict = {}


def get_runner(reps: int = 1, loop: bool = False,
               donate: bool = True) -> CachedRunner:
    key = (reps, loop, donate)
    if key not in _RUNNER_CACHE:
        _RUNNER_CACHE[key] = CachedRunner(build_kernel(reps, loop), donate)
    return _RUNNER_CACHE[key]


def kernel(all_encoder_layers, input_mask, token_map, fc_w, fc_b, layer_index):
    in_maps = _host_prep(all_encoder_layers, token_map, fc_w, fc_b, layer_index)
    runner = get_runner(1)
    out = runner(in_maps)
    idx = {n: i for i, n in enumerate(runner.out_names)}
    rep = _unpack_out(out[idx["rep"]])
    ote = _unpack_out(out[idx["ote"]])
    return rep, ote
